# revision 36
# baseline (speedup 1.0000x reference)
"""Causal self-attention (B=2, T=2048, C=1024, H=16) on 8 Trainium2 NeuronCores.

Sharding (Megatron-style, chosen per hint): core c handles batch b = c//4 and
head group g = c%4 (4 heads each).  c_attn is column-parallel (each core gets
the 3x256 q/k/v columns for its heads), c_proj is row-parallel (each core gets
the 256 rows for its heads) and the 4 partial outputs per batch are summed on
the host (the row-parallel all-reduce), plus b_proj.

Per-core dataflow (all matmuls fp32r: full-rate fp32-format PE mode):
  x^T [C, T] is pre-transposed on host, so contraction dims always sit on
  SBUF partitions:
   1. QT/KT [d, t] (d-major, head pairs packed on 128 partitions) and
      V [t, d] (t-major, with a ones column appended -> fused softmax denom).
   2. ST tile [j, i] = (K Q^T) per head, causal-windowed; triangular additive
      mask on diagonal 128-blocks; exp on ScalarE with fused 1/sqrt(64) scale.
   3. YT [e, i] += V_aug^T @ P accumulated over j-blocks in PSUM; row 64/63 is
      the softmax denominator (ones column of V_aug).
   4. Normalize by broadcast reciprocal, then out[i, :] = sum_ho YT^T @ W_proj.
"""

import os
import sys
import types
from contextlib import ExitStack

import ml_dtypes
import numpy as np

for _p in ("/opt/trn_rl_repo",):
    if os.path.isdir(_p) and _p not in sys.path:
        sys.path.append(_p)
os.environ.setdefault("JAX_PLATFORMS", "cpu")

import concourse.bass as bass
import concourse.tile as tile
from concourse import bacc, mybir
from concourse.bass_utils import run_bass_kernel_spmd

B, T, C, H = 2, 2048, 1024, 16
P = 128
CO = C // P          # 8 contraction blocks for the qkv projection
HL = H // 4          # 4 local heads per core
D = C // H           # 64
NEG = -1.0e30
F32 = mybir.dt.float32
F32R = mybir.dt.float32r
BF16 = mybir.dt.bfloat16
EXPF = mybir.ActivationFunctionType.Exp
ADD = mybir.AluOpType.add
MULT = mybir.AluOpType.mult

_CACHE = {}


def _install_ntff_hook():
    """Agent image's antenv lacks axon_hooks; recreate so trace=True works."""
    try:
        from antenv import axon_hooks  # noqa: F401
        return
    except ImportError:
        pass
    try:
        import antenv
        from trn_agent_boot.trn_boot import _ntff_profile_via_ctypes
    except ImportError:
        return
    mod = types.ModuleType("antenv.axon_hooks")
    _hook = [None]
    mod.set_axon_ntff_profile_hook = lambda h: _hook.__setitem__(0, h)
    mod.get_axon_ntff_profile_hook = lambda: _hook[0]
    sys.modules["antenv.axon_hooks"] = mod
    antenv.axon_hooks = mod
    so = "/opt/axon/libaxon_pjrt.so"
    if os.path.exists(so):
        mod.set_axon_ntff_profile_hook(_ntff_profile_via_ctypes(so))


def build_module():
    nc = bacc.Bacc("TRN2", target_bir_lowering=False, debug=False, num_devices=8)

    xt_d = nc.dram_tensor("xt", [C, T], BF16, kind="ExternalInput").ap()
    wq_d = nc.dram_tensor("wq", [C, 256], BF16, kind="ExternalInput").ap()
    wk_d = nc.dram_tensor("wk", [C, 256], BF16, kind="ExternalInput").ap()
    wv_d = nc.dram_tensor("wv", [C, 256], BF16, kind="ExternalInput").ap()
    wp_d = nc.dram_tensor("wp", [256, C], BF16, kind="ExternalInput").ap()
    bq_d = nc.dram_tensor("bq", [256], F32, kind="ExternalInput").ap()
    bk_d = nc.dram_tensor("bk", [256], F32, kind="ExternalInput").ap()
    bv_d = nc.dram_tensor("bv", [256], F32, kind="ExternalInput").ap()
    tri_d = nc.dram_tensor("tri", [P, P], F32, kind="ExternalInput").ap()
    ones_d = nc.dram_tensor("onesd", [T // P * HL], BF16, kind="ExternalInput").ap()
    out_d = nc.dram_tensor("out", [T, C], F32, kind="ExternalOutput").ap()

    with tile.TileContext(nc) as tc, ExitStack() as ctx:
        const = ctx.enter_context(tc.tile_pool(name="const", bufs=1))
        s1w = ctx.enter_context(tc.tile_pool(name="s1w", bufs=1))
        # PSUM: 8 banks of [128, 512]f32 total.  acc(2) + stp(3) + ytp(3).
        psA = ctx.enter_context(tc.tile_pool(name="psA", bufs=3, space="PSUM"))
        psS = ctx.enter_context(tc.tile_pool(name="psS", bufs=3, space="PSUM"))
        psY = ctx.enter_context(tc.tile_pool(name="psY", bufs=2, space="PSUM"))
        ppool = ctx.enter_context(tc.tile_pool(name="ppool", bufs=6))
        rpool = ctx.enter_context(tc.tile_pool(name="rpool", bufs=4))
        opool = ctx.enter_context(tc.tile_pool(name="opool", bufs=3))
        dpool = ctx.enter_context(tc.tile_pool(name="dpool", bufs=4, space="DRAM"))

        # ---- persistent SBUF tensors -------------------------------------
        qt = const.tile([P, 2, T], BF16, tag="qt")     # [d, do, t]; head pair per do
        kt = const.tile([P, 2, T], BF16, tag="kt")
        vsb = const.tile([P, T // P, HL, 66], BF16, tag="vsb")  # [tp, to, l, 1|V|1]
        yt2 = const.tile([P, 2, T], BF16, tag="yt2")   # Y^T (unnorm, then scaled)
        wp_sb = const.tile([P, 2, C], BF16, tag="wp")
        tri_sb = const.tile([P, P], F32, tag="tri")
        bq_sb = const.tile([P, 2], F32, tag="bq")
        bk_sb = const.tile([P, 2], F32, tag="bk")
        bv_sb = const.tile([P, 256], F32, tag="bv")

        xt_sb = s1w.tile([P, CO, T], BF16, tag="xt")
        wq_sb = s1w.tile([P, CO, 256], BF16, tag="wq")
        wk_sb = s1w.tile([P, CO, 256], BF16, tag="wk")
        wv_sb = s1w.tile([P, CO, 256], BF16, tag="wv")

        # ---- input DMA (split per contraction block so PE can start early)
        xt_r = xt_d.rearrange("(co p) t -> p co t", p=P)
        wq_r = wq_d.rearrange("(co p) d -> p co d", p=P)
        wk_r = wk_d.rearrange("(co p) d -> p co d", p=P)
        wv_r = wv_d.rearrange("(co p) d -> p co d", p=P)
        for co in range(CO):
            nc.sync.dma_start(wq_sb[:, co], wq_r[:, co])
            nc.sync.dma_start(wk_sb[:, co], wk_r[:, co])
            nc.sync.dma_start(wv_sb[:, co], wv_r[:, co])
            nc.sync.dma_start(xt_sb[:, co], xt_r[:, co])
        nc.sync.dma_start(wp_sb[:], wp_d.rearrange("(ho p) n -> p ho n", p=P))
        nc.sync.dma_start(tri_sb[:], tri_d)
        nc.sync.dma_start(bq_sb[:], bq_d.rearrange("(do p) -> p do", p=P))
        nc.sync.dma_start(bk_sb[:], bk_d.rearrange("(do p) -> p do", p=P))
        nc.sync.dma_start(
            bv_sb[:],
            bass.AP(tensor=bv_d.tensor, offset=bv_d.offset,
                    ap=[[0, P]] + list(bv_d.ap)),
        )
        nc.vector.memset(vsb[:, :, :, 65:66], 1.0)

        # ---- stage 1: qkv projection -------------------------------------
        # QT/KT d-major: psum[d, t] = W[:, dcols]^T @ x^T
        for w_sb, b_sb, dst in ((wq_sb, bq_sb, qt), (wk_sb, bk_sb, kt)):
            for do in range(2):
                for t4 in range(T // 512):
                    ps = psA.tile([P, 512], F32, tag="acc")
                    for co in range(CO):
                        nc.tensor.matmul(
                            ps[:],
                            lhsT=w_sb[:, co, do * P:(do + 1) * P],
                            rhs=xt_sb[:, co, t4 * 512:(t4 + 1) * 512],
                            start=(co == 0), stop=(co == CO - 1),
                        )
                    nc.vector.tensor_scalar_add(
                        dst[:, do, t4 * 512:(t4 + 1) * 512], ps[:], b_sb[:, do:do + 1])
        # V t-major: psum[t, d] = x^T-block^T @ Wv
        for to in range(T // P):
            ps = psA.tile([P, 512], F32, tag="acc", name="vps")[:, 0:256]
            for co in range(CO):
                nc.tensor.matmul(
                    ps[:],
                    lhsT=xt_sb[:, co, to * P:(to + 1) * P],
                    rhs=wv_sb[:, co, :],
                    start=(co == 0), stop=(co == CO - 1),
                )
            nc.vector.tensor_tensor(
                vsb[:, to, :, 1:65],
                ps[:].rearrange("p (l e) -> p l e", l=HL),
                bv_sb[:].rearrange("p (l e) -> p l e", l=HL),
                op=ADD,
            )

        # ---- stages 2-4: attention, head pair (2*ho, 2*ho+1) -------------
        NB = T // 512                       # 4 i-blocks of 512
        for ho in range(2):
            for ib in range(NB):
                ytp = [psY.tile([P, 512], F32, tag="ytp", name=f"ytp_{hp}")
                       for hp in range(2)]
                njb = 4 * ib + 4

                def win(jb):
                    r = jb - 4 * ib
                    i0 = jb * P if r >= 0 else ib * 512
                    return r, i0, (ib + 1) * 512 - i0

                pts = {}

                def emit_st(jb):
                    r, i0, N = win(jb)
                    jsl = slice(jb * P, (jb + 1) * P)
                    pair = []
                    for hp in range(2):
                        pb = hp * 64
                        stp = psS.tile([P, 512], F32, tag="stp")
                        nc.tensor.matmul(
                            stp[:, :N], lhsT=kt[pb:pb + 64, ho, jsl],
                            rhs=qt[pb:pb + 64, ho, i0:i0 + N],
                            start=True, stop=True)
                        if r >= 0:
                            nc.vector.tensor_tensor(
                                stp[:, 0:P], stp[:, 0:P], tri_sb[:], op=ADD)
                        pt = ppool.tile([P, 512], BF16, tag="pt")
                        nc.scalar.activation(pt[:, :N], stp[:, :N], EXPF,
                                             scale=float(1.0 / np.sqrt(D)))
                        pair.append(pt)
                    pts[jb] = pair

                def emit_yt(jb):
                    _, i0, N = win(jb)
                    f0 = i0 - ib * 512
                    last = jb == njb - 1
                    pair = pts.pop(jb)
                    for hp in range(2):
                        nc.tensor.matmul(
                            ytp[hp][0:65, f0:f0 + N],
                            lhsT=vsb[:, jb, 2 * ho + hp, 1:66],
                            rhs=pair[hp][:, :N], start=(jb == 0), stop=last)

                # software pipeline: keep PE two ST-pairs ahead of the
                # exp-dependent YT accumulations so it never idles on ScalarE
                emit_st(0)
                if njb > 1:
                    emit_st(1)
                for jb in range(njb):
                    if jb + 2 < njb:
                        emit_st(jb + 2)
                    emit_yt(jb)
                # epilogue: stage unnormalized Y + denominator rows, free PSUM
                iw = slice(ib * 512, (ib + 1) * 512)
                den2 = rpool.tile([2, 512], F32, tag="den2")
                for hp in range(2):
                    nc.vector.tensor_copy(
                        yt2[hp * 64:hp * 64 + 64, ho, iw], ytp[hp][0:64, :])
                    dr = rpool.tile([1, 512], F32, tag="dr", name=f"dr{hp}")
                    nc.vector.tensor_copy(dr[:], ytp[hp][64:65, :])
                    nc.sync.dma_start(den2[hp:hp + 1, :], dr[:])
                # per-block reciprocal + broadcast normalize (off PE path)
                rden2 = rpool.tile([2, 512], F32, tag="rden2")
                nc.vector.reciprocal(rden2[:], den2[:])
                dscr = dpool.tile([2, 512], F32, tag="dscr")
                nc.sync.dma_start(dscr[:], rden2[:])
                for hp in range(2):
                    src = dscr[hp, :]
                    pb = hp * 64
                    rdb = rpool.tile([P, 512], F32, tag="rdb")
                    nc.sync.dma_start(
                        rdb[pb:pb + 64, :],
                        bass.AP(tensor=src.tensor, offset=src.offset,
                                ap=[[0, 64]] + list(src.ap)))
                    ysl = yt2[pb:pb + 64, ho, iw]
                    nc.vector.tensor_tensor(ysl, ysl, rdb[pb:pb + 64, :], op=MULT)

        # ---- stage 5: output projection (row-parallel partial) -----------
        for i1 in range(T // P):
            isl = slice(i1 * P, (i1 + 1) * P)
            for nh in range(C // 512):
                nsl = slice(nh * 512, (nh + 1) * 512)
                ps = psA.tile([P, 512], F32, tag="acc")
                for ho in range(2):
                    nc.tensor.matmul(
                        ps[:], lhsT=yt2[:, ho, isl], rhs=wp_sb[:, ho, nsl],
                        start=(ho == 0), stop=(ho == 1))
                ot = opool.tile([P, 512], F32, tag="ot")
                nc.vector.tensor_copy(ot[:], ps[:])
                nc.sync.dma_start(out_d[isl, nsl], ot[:])

    nc.compile()
    return nc


def _get_module():
    if "nc" not in _CACHE:
        _CACHE["nc"] = build_module()
    return _CACHE["nc"]


def _make_in_maps(x, W_attn, b_attn, W_proj):
    tri = np.where(np.arange(P)[None, :] >= np.arange(P)[:, None],
                   np.float32(0.0), np.float32(NEG)).astype(np.float32)
    bf = ml_dtypes.bfloat16
    in_maps = []
    for core in range(8):
        b, g = divmod(core, 4)
        cs = slice(g * 256, (g + 1) * 256)
        in_maps.append({
            "xt": np.ascontiguousarray(x[b].T).astype(bf),
            "wq": np.ascontiguousarray(W_attn[:, g * 256:(g + 1) * 256]).astype(bf),
            "wk": np.ascontiguousarray(
                W_attn[:, C + g * 256:C + (g + 1) * 256]).astype(bf),
            "wv": np.ascontiguousarray(
                W_attn[:, 2 * C + g * 256:2 * C + (g + 1) * 256]).astype(bf),
            "wp": np.ascontiguousarray(W_proj[cs, :]).astype(bf),
            "bq": np.ascontiguousarray(b_attn[cs]),
            "bk": np.ascontiguousarray(b_attn[C + g * 256:C + (g + 1) * 256]),
            "bv": np.ascontiguousarray(b_attn[2 * C + g * 256:2 * C + (g + 1) * 256]),
            "tri": tri,
            "onesd": np.ones(T // P * HL, bf),
        })
    return in_maps


def _gather(results, b_proj):
    y = np.empty((B, T, C), np.float32)
    for b in range(B):
        acc = results[4 * b]["out"].astype(np.float32).copy()
        for g in range(1, 4):
            acc += results[4 * b + g]["out"]
        y[b] = acc + b_proj[None, :].astype(np.float32)
    return y


def kernel(x, W_attn, b_attn, W_proj, b_proj, _trace=False):
    x = np.asarray(x, np.float32)
    W_attn = np.asarray(W_attn, np.float32)
    b_attn = np.asarray(b_attn, np.float32)
    W_proj = np.asarray(W_proj, np.float32)
    b_proj = np.asarray(b_proj, np.float32)

    nc = _get_module()
    in_maps = _make_in_maps(x, W_attn, b_attn, W_proj)
    kw = {}
    if _trace:
        _install_ntff_hook()
        kw = dict(trace=True)
    res = run_bass_kernel_spmd(nc, in_maps, core_ids=list(range(8)), **kw)
    out = _gather(res.results, b_proj)
    if _trace:
        return out, res
    return out
